# revision 5
# baseline (speedup 1.0000x reference)
"""Hamming-distance embedding kernel for Trainium2 (8 NeuronCores, SPMD).

Math: for binary x in {0,1}^(B,L), refs in {0,1}^(D,L):
    hamming[b,d]   = sum_x[b] + sum_r[d] - 2*dot[b,d]
    out            = (hamming - L/2) / (0.5*sqrt(L))
Substituting a = 2x-1, c = 1-2r (both in {-1,+1}):
    hamming - L/2  = 0.5 * sum_l a[b,l]*c[d,l]
    out[b,d]       = (a @ c^T)[b,d] / sqrt(L)
So the whole module is ONE {+-1} matmul with a scalar scale. The +-1 values
are exact in bf16 and the f32 PSUM accumulation of +-1 products is exact.

Sharding: data-parallel over batch; refs replicated. Host side transposes
both operands (pure relayout, dtypes preserved) so the contraction dim L is
the SBUF partition dim for both matmul operands with contiguous DMA rows.

Raw bass (no TileContext): this container's walrus rejects instructions
with more than a couple of attached sync waits, which Tile's kernel-tail
drain always exceeds. Raw engine blocks with one explicit wait_ge per
dependency stay within the budget. Per-chunk DMA semaphores (threshold 16)
make waits order-independent across HWDGE queues.

Pipeline per core:
  SP   : issue all 32 input-chunk DMAs up front; drain output DMAs at tail
  ACT  : a_bf[li] = bf16(2*x - 1)         (int32 -> bf16 affine cast)
  DVE  : c_bf[li] = bf16(1 - 2*r)         (f32 -> bf16 affine cast)
         out_sb[bi] = psum[bi] * 1/sqrt(L) (after accumulation done)
  PE   : psum[bi][dh] += a_bf[li][:,bi].T @ c_bf[li][:,dh]  (16 l-steps)
"""

import math
from contextlib import ExitStack

import numpy as np

import concourse.bass as bass
import concourse.mybir as mybir
from concourse.bass_utils import run_bass_kernel_spmd

N_CORES = 8
B, D, L = 4096, 1024, 2048
B_SHARD = B // N_CORES  # 512

P = 128          # SBUF partitions / matmul contraction tile
N_TILE = 512     # matmul free-dim tile (one PSUM bank of f32)


def build_nc(b_shard: int = B_SHARD, d: int = D, l_dim: int = L) -> bass.Bass:
    l_chunks = l_dim // P
    b_chunks = b_shard // P
    d_halves = d // N_TILE
    scale = 1.0 / math.sqrt(l_dim)

    nc = bass.Bass()
    xT = nc.declare_dram_parameter("xT", [l_dim, b_shard], mybir.dt.int32, isOutput=False)
    refsT = nc.declare_dram_parameter("refsT", [l_dim, d], mybir.dt.float32, isOutput=False)
    out = nc.declare_dram_parameter("out", [b_shard, d], mybir.dt.float32, isOutput=True)

    with ExitStack() as ctx:
        xt_raw = [ctx.enter_context(nc.sbuf_tensor(f"xt{i}", [P, b_shard], mybir.dt.int32))
                  for i in range(l_chunks)]
        rt_raw = [ctx.enter_context(nc.sbuf_tensor(f"rt{i}", [P, d], mybir.dt.float32))
                  for i in range(l_chunks)]
        a_bf = [ctx.enter_context(nc.sbuf_tensor(f"ab{i}", [P, b_shard], mybir.dt.bfloat16))
                for i in range(l_chunks)]
        c_bf = [ctx.enter_context(nc.sbuf_tensor(f"cb{i}", [P, d], mybir.dt.bfloat16))
                for i in range(l_chunks)]
        out_sb = [ctx.enter_context(nc.sbuf_tensor(f"os{i}", [P, d], mybir.dt.float32))
                  for i in range(b_chunks)]
        psum = [[ctx.enter_context(nc.psum_tensor(f"pm{bi}_{dh}", [P, N_TILE], mybir.dt.float32))
                 for dh in range(d_halves)] for bi in range(b_chunks)]

        sem_x = [ctx.enter_context(nc.semaphore(f"sx{i}")) for i in range(l_chunks)]
        sem_r = [ctx.enter_context(nc.semaphore(f"sr{i}")) for i in range(l_chunks)]
        sem_a = ctx.enter_context(nc.semaphore("sa"))
        sem_c = ctx.enter_context(nc.semaphore("sc"))
        sem_mm = ctx.enter_context(nc.semaphore("smm"))
        sem_cp = ctx.enter_context(nc.semaphore("scp"))
        sem_out = ctx.enter_context(nc.semaphore("so"))

        with nc.Block() as block:

            @block.sync
            def _(sync):
                for li in range(l_chunks):
                    sync.dma_start(out=xt_raw[li][:], in_=xT[li * P:(li + 1) * P, :]
                                   ).then_inc(sem_x[li], 16)
                    sync.dma_start(out=rt_raw[li][:], in_=refsT[li * P:(li + 1) * P, :]
                                   ).then_inc(sem_r[li], 16)
                for bi in range(b_chunks):
                    sync.wait_ge(sem_cp, bi + 1)
                    sync.dma_start(out=out[bi * P:(bi + 1) * P, :], in_=out_sb[bi][:]
                                   ).then_inc(sem_out, 16)
                sync.wait_ge(sem_out, 16 * b_chunks)

            @block.scalar
            def _(scalar):
                for li in range(l_chunks):
                    scalar.wait_ge(sem_x[li], 16)
                    nc.scalar.activation(
                        a_bf[li][:], xt_raw[li][:],
                        mybir.ActivationFunctionType.Copy, bias=-1.0, scale=2.0,
                    ).then_inc(sem_a, 1)

            @block.vector
            def _(vector):
                for li in range(l_chunks):
                    vector.wait_ge(sem_r[li], 16)
                    nc.vector.tensor_scalar(
                        out=c_bf[li][:], in0=rt_raw[li][:],
                        scalar1=-2.0, scalar2=1.0,
                        op0=mybir.AluOpType.mult, op1=mybir.AluOpType.add,
                    ).then_inc(sem_c, 1)
                for bi in range(b_chunks):
                    vector.wait_ge(sem_mm, d_halves * (bi + 1))
                    for dh in range(d_halves):
                        ins = nc.vector.tensor_scalar_mul(
                            out_sb[bi][:, dh * N_TILE:(dh + 1) * N_TILE],
                            psum[bi][dh][:], scale)
                    ins.then_inc(sem_cp, 1)

            @block.tensor
            def _(tensor):
                for li in range(l_chunks):
                    tensor.wait_ge(sem_a, li + 1)
                    tensor.wait_ge(sem_c, li + 1)
                    for bi in range(b_chunks):
                        for dh in range(d_halves):
                            mm = nc.tensor.matmul(
                                psum[bi][dh][:],
                                lhsT=a_bf[li][:, bi * P:(bi + 1) * P],
                                rhs=c_bf[li][:, dh * N_TILE:(dh + 1) * N_TILE],
                                start=(li == 0),
                                stop=(li == l_chunks - 1),
                            )
                            if li == l_chunks - 1:
                                mm.then_inc(sem_mm, 1)

    return nc


_NC_CACHE: dict = {}


def kernel(x: np.ndarray, references: np.ndarray) -> np.ndarray:
    assert x.shape == (B, L) and references.shape == (D, L)
    xT = np.ascontiguousarray(x.T)                    # (L, B) int32
    refsT = np.ascontiguousarray(references.T)        # (L, D) float32

    in_maps = [
        {
            "xT": np.ascontiguousarray(xT[:, c * B_SHARD:(c + 1) * B_SHARD]),
            "refsT": refsT,
        }
        for c in range(N_CORES)
    ]

    if "nc" not in _NC_CACHE:
        _NC_CACHE["nc"] = build_nc()
    nc = _NC_CACHE["nc"]

    res = run_bass_kernel_spmd(nc, in_maps, core_ids=list(range(N_CORES)))
    outs = [res.results[c]["out"] for c in range(N_CORES)]
    return np.ascontiguousarray(np.concatenate(outs, axis=0), dtype=np.float32)
